# revision 4
# baseline (speedup 1.0000x reference)
"""Causal self-attention (B=4, T=2048, C=768, H=6, D=128) on 8 trn2 NeuronCores.

Sharding: 24 (batch, head) units -> 8 cores, each core owns 1 batch x 3 heads.
Per core: QKV projections for its 3 heads, RoPE + per-head norm, causal
attention, partial output projection over its heads' columns.
Unshard: out[b] = partial[core 2b] + partial[core 2b+1]  (tensor-parallel sum).

Device-side layout choices:
  - Q,K computed in [t, d] layout (rope/norm are free-dim ops), then
    PE-transposed to [d, t] so the scores matmul contracts d on partitions.
  - scores computed TRANSPOSED: sT[s, q] = K[s]:Q[q] so that the AV matmul
    (contraction over s) can consume exp(sT) directly with V in natural [s, d]
    layout; output y arrives as yT[d, q], which is exactly the lhsT layout the
    output projection needs.  No attention-matrix transposes anywhere.
  - softmax denominator: exp tiles accumulated on DVE, partition-summed with a
    ones-vector matmul on PE, reciprocal on DVE, partition-broadcast on GPSIMD.
  - no max-subtraction in softmax: q,k are unit-normalized so |score| <= 11.4
    and exp stays comfortably inside fp32 range (matches reference softmax
    bit-for-bit up to rounding).
  - causal mask inside diagonal blocks: affine_select (iota predicate) zeroes
    exp values where s > q; fully-masked-out blocks are simply never computed.
"""

import numpy as np

import concourse.bacc as bacc
import concourse.bass as bass
import concourse.mybir as mybir
from concourse import tile
from concourse.bass_utils import run_bass_kernel_spmd

F32 = mybir.dt.float32
AF = mybir.ActivationFunctionType
ALU = mybir.AluOpType

B, T, C, H, D = 4, 2048, 768, 6, 128
HALF = D // 2
NH = 3            # heads per core
CT = C // 128     # 6 contraction tiles for projections
NT = T // 128     # 16 token tiles
QC = 512          # query-chunk width for attention
NQC = T // QC     # 4 chunks
SCALE = 1.0 / float(np.sqrt(D))
EPS = 1e-6

_CACHE = {}


def _build_nc():
    nc = bacc.Bacc("TRN2")

    xT = nc.dram_tensor("xT", [C, T], F32, kind="ExternalInput")
    wqT = nc.dram_tensor("wqT", [C, NH * D], F32, kind="ExternalInput")
    wkT = nc.dram_tensor("wkT", [C, NH * D], F32, kind="ExternalInput")
    wvT = nc.dram_tensor("wvT", [C, NH * D], F32, kind="ExternalInput")
    wpT = nc.dram_tensor("wpT", [NH * D, C], F32, kind="ExternalInput")
    cos3 = nc.dram_tensor("cos3", [T, NH * HALF], F32, kind="ExternalInput")
    sin3 = nc.dram_tensor("sin3", [T, NH * HALF], F32, kind="ExternalInput")
    ident = nc.dram_tensor("ident", [128, 128], F32, kind="ExternalInput")
    out = nc.dram_tensor("out", [T, C], F32, kind="ExternalOutput")

    with tile.TileContext(nc) as tc:
        with (
            tc.tile_pool(name="persist", bufs=1) as persist,
            tc.tile_pool(name="qkvbuf", bufs=1) as qkvbuf,
            tc.tile_pool(name="psA", bufs=4, space="PSUM") as psA,
            tc.tile_pool(name="psY", bufs=2, space="PSUM") as psY,
            tc.tile_pool(name="psT", bufs=2, space="PSUM") as psT,
        ):
            QT = qkvbuf.tile([128, NH, T], F32)       # [d, h, t]
            KT = qkvbuf.tile([128, NH, T], F32)       # [d, h, t]
            V = qkvbuf.tile([128, NT, NH * D], F32)   # [s%128, s//128, h*D+d]
            ones = persist.tile([128, 1], F32)
            nc.gpsimd.memset(ones[:], 1.0)
            idn = persist.tile([128, 128], F32)
            nc.sync.dma_start(idn[:], ident[:])
            wp_sb = persist.tile([128, NH, C], F32)   # [d, h, c]
            nc.sync.dma_start(wp_sb[:], wpT.rearrange("(h p) c -> p h c", p=128))

            # ---------------- stage 1+2: QKV projection + rope + norm ---------
            with (
                tc.tile_pool(name="wbuf", bufs=1) as wbuf,
                tc.tile_pool(name="xch", bufs=3) as xpool,
                tc.tile_pool(name="rope", bufs=3) as rpool,
                tc.tile_pool(name="stat", bufs=6) as spool,
            ):
                wq_sb = wbuf.tile([128, CT, NH * D], F32)
                wk_sb = wbuf.tile([128, CT, NH * D], F32)
                wv_sb = wbuf.tile([128, CT, NH * D], F32)
                nc.sync.dma_start(wq_sb[:], wqT.rearrange("(ci p) o -> p ci o", p=128))
                nc.sync.dma_start(wk_sb[:], wkT.rearrange("(ci p) o -> p ci o", p=128))
                nc.sync.dma_start(wv_sb[:], wvT.rearrange("(ci p) o -> p ci o", p=128))
                cos_sb = wbuf.tile([128, NT, NH * HALF], F32)
                sin_sb = wbuf.tile([128, NT, NH * HALF], F32)
                nc.sync.dma_start(cos_sb[:], cos3.rearrange("(tt p) f -> p tt f", p=128))
                nc.sync.dma_start(sin_sb[:], sin3.rearrange("(tt p) f -> p tt f", p=128))

                xT_r = xT.rearrange("(ci p) (tt t) -> p ci tt t", p=128, t=128)

                for tt in range(NT):
                    xch = xpool.tile([128, CT, 128], F32, tag="xch")
                    nc.sync.dma_start(xch[:], xT_r[:, :, tt, :])

                    qps = psA.tile([128, NH * D], F32, tag="ps")
                    kps = psA.tile([128, NH * D], F32, tag="ps")
                    vps = psA.tile([128, NH * D], F32, tag="ps")
                    for ci in range(CT):
                        st_, sp_ = (ci == 0), (ci == CT - 1)
                        lhs = xch[:, ci, :]
                        nc.tensor.matmul(qps[:], lhs, wq_sb[:, ci, :], start=st_, stop=sp_)
                        nc.tensor.matmul(kps[:], lhs, wk_sb[:, ci, :], start=st_, stop=sp_)
                        nc.tensor.matmul(vps[:], lhs, wv_sb[:, ci, :], start=st_, stop=sp_)

                    # V: straight copy PSUM -> SBUF in natural [t, o] layout
                    nc.scalar.copy(V[:, tt, :], vps[:])

                    cos_t = cos_sb[:, tt].rearrange("p (h f) -> p h f", h=NH)
                    sin_t = sin_sb[:, tt].rearrange("p (h f) -> p h f", h=NH)

                    for ps, dstT in ((qps, QT), (kps, KT)):
                        ps_v = ps[:].rearrange("p (h d) -> p h d", h=NH)
                        a = ps_v[:, :, 0:HALF]      # x1  [128, 3, 64]
                        b = ps_v[:, :, HALF:D]      # x2  [128, 3, 64]
                        r = rpool.tile([128, NH * D], F32, tag="r")
                        r_v = r[:].rearrange("p (h d) -> p h d", h=NH)
                        r1 = r_v[:, :, 0:HALF]
                        r2 = r_v[:, :, HALF:D]
                        tbs = rpool.tile([128, NH * HALF], F32, tag="tbs")
                        tas = rpool.tile([128, NH * HALF], F32, tag="tas")
                        tbs_v = tbs[:].rearrange("p (h f) -> p h f", h=NH)
                        tas_v = tas[:].rearrange("p (h f) -> p h f", h=NH)
                        # rope: r1 = a*cos + b*sin ; r2 = b*cos - a*sin
                        nc.vector.tensor_mul(tbs_v, b, sin_t)
                        nc.vector.tensor_mul(tas_v, a, sin_t)
                        nc.vector.tensor_mul(r1, a, cos_t)
                        nc.vector.tensor_mul(r2, b, cos_t)
                        nc.vector.tensor_add(r1, r1, tbs_v)
                        nc.vector.tensor_sub(r2, r2, tas_v)

                        # norm over d (free dim), per head, ddof=1:
                        # rstd = 1/(sqrt((sumsq - mean*sum)/127) + eps)
                        sums = spool.tile([128, NH], F32, tag="sums")
                        nc.vector.tensor_reduce(sums[:], r_v, axis=mybir.AxisListType.X, op=ALU.add)
                        sumsq = spool.tile([128, NH], F32, tag="sumsq")
                        sq = rpool.tile([128, NH * D], F32, tag="sq")
                        for h in range(NH):
                            nc.scalar.activation(
                                sq[:, h * D:(h + 1) * D], r[:, h * D:(h + 1) * D],
                                AF.Square, accum_out=sumsq[:, h:h + 1],
                            )
                        mean = spool.tile([128, NH], F32, tag="mean")
                        nc.scalar.mul(mean[:], sums[:], 1.0 / D)
                        var = spool.tile([128, NH], F32, tag="var")
                        nc.vector.tensor_mul(var[:], mean[:], sums[:])
                        nc.vector.tensor_sub(var[:], sumsq[:], var[:])
                        stdv = spool.tile([128, NH], F32, tag="stdv")
                        nc.scalar.activation(stdv[:], var[:], AF.Sqrt, scale=1.0 / (D - 1))
                        nc.vector.tensor_scalar_add(stdv[:], stdv[:], EPS)
                        rstd = spool.tile([128, NH], F32, tag="rstd")
                        nc.vector.reciprocal(rstd[:], stdv[:])
                        mrs = spool.tile([128, NH], F32, tag="mrs")
                        nc.vector.tensor_mul(mrs[:], mean[:], rstd[:])
                        nrm = rpool.tile([128, NH * D], F32, tag="nrm")
                        for h in range(NH):
                            # (r - mean)*rstd = r*rstd - mean*rstd
                            nc.vector.tensor_scalar(
                                nrm[:, h * D:(h + 1) * D], r[:, h * D:(h + 1) * D],
                                rstd[:, h:h + 1], mrs[:, h:h + 1],
                                op0=ALU.mult, op1=ALU.subtract,
                            )
                        for h in range(NH):
                            tps = psT.tile([128, 128], F32, tag="tp")
                            nc.tensor.transpose(tps[:], nrm[:, h * D:(h + 1) * D], idn[:])
                            nc.scalar.copy(dstT[:, h, tt * 128:(tt + 1) * 128], tps[:])

            # ---------------- stage 3+4: attention + output projection --------
            with (
                tc.tile_pool(name="att", bufs=3) as apool,
                tc.tile_pool(name="acc", bufs=2) as accpool,
                tc.tile_pool(name="ybuf", bufs=2) as ypool,
                tc.tile_pool(name="obuf", bufs=3) as opool,
            ):
                out_r = out.rearrange("(tt p) c -> p tt c", p=128)
                for qc in range(NQC):
                    Q0 = qc * QC
                    n_st = (Q0 + QC) // 128
                    yTc = ypool.tile([128, NH, QC], F32, tag="yT")  # [d, h, q]
                    for h in range(NH):
                        yps = psY.tile([128, QC], F32, tag="yps")
                        dacc = accpool.tile([128, QC], F32, tag="dacc")
                        for st in range(n_st):
                            sps = psA.tile([128, QC], F32, tag="ps")
                            nc.tensor.matmul(
                                sps[:],
                                KT[:, h, st * 128:(st + 1) * 128],
                                QT[:, h, Q0:Q0 + QC],
                                start=True, stop=True,
                            )
                            et = apool.tile([128, QC], F32, tag="et")
                            nc.scalar.activation(et[:], sps[:], AF.Exp, scale=SCALE)
                            if st * 128 >= Q0:  # diagonal block: zero where s > q
                                k = st - n_st + 4
                                nc.gpsimd.affine_select(
                                    et[:], et[:],
                                    pattern=[[1, QC]],
                                    compare_op=ALU.is_ge,
                                    fill=0.0,
                                    base=-(128 * k),
                                    channel_multiplier=-1,
                                )
                            if st == 0:
                                nc.vector.tensor_copy(dacc[:], et[:])
                            else:
                                nc.vector.tensor_add(dacc[:], dacc[:], et[:])
                            nc.tensor.matmul(
                                yps[:],
                                V[:, st, h * D:(h + 1) * D],
                                et[:],
                                start=(st == 0), stop=(st == n_st - 1),
                                skip_group_check=True,
                            )
                        dps = psT.tile([128, QC], F32, tag="tp")
                        nc.tensor.matmul(dps[:1, :], ones[:], dacc[:], start=True, stop=True)
                        rc = accpool.tile([128, QC], F32, tag="rc")
                        nc.vector.reciprocal(rc[:1, :], dps[:1, :])
                        rcb = accpool.tile([128, QC], F32, tag="rcb")
                        nc.gpsimd.partition_broadcast(rcb[:], rc[:1, :])
                        nc.vector.tensor_mul(yTc[:, h, :], yps[:], rcb[:])

                    # output projection for this chunk's 4 token tiles
                    for j in range(QC // 128):
                        tt = qc * (QC // 128) + j
                        op0 = psA.tile([128, C // 2], F32, tag="ps")
                        op1 = psA.tile([128, C // 2], F32, tag="ps")
                        for h in range(NH):
                            lhs = yTc[:, h, j * 128:(j + 1) * 128]
                            nc.tensor.matmul(op0[:], lhs, wp_sb[:, h, 0:C // 2],
                                             start=(h == 0), stop=(h == NH - 1))
                            nc.tensor.matmul(op1[:], lhs, wp_sb[:, h, C // 2:C],
                                             start=(h == 0), stop=(h == NH - 1))
                        ot = opool.tile([128, C], F32, tag="ot")
                        nc.scalar.copy(ot[:, 0:C // 2], op0[:])
                        nc.scalar.copy(ot[:, C // 2:C], op1[:])
                        nc.sync.dma_start(out_r[:, tt, :], ot[:])

    nc.compile()
    return nc


def _get_nc():
    if "nc" not in _CACHE:
        _CACHE["nc"] = _build_nc()
    return _CACHE["nc"]


def _in_maps(x, cos, sin, wq, wk, wv, wproj):
    cos3 = np.ascontiguousarray(np.tile(np.asarray(cos, np.float32), (1, NH)))
    sin3 = np.ascontiguousarray(np.tile(np.asarray(sin, np.float32), (1, NH)))
    ident = np.eye(128, dtype=np.float32)
    maps = []
    for c in range(8):
        b = c // 2
        hs = (c % 2) * NH
        sl = slice(hs * D, (hs + NH) * D)
        maps.append({
            "xT": np.ascontiguousarray(np.asarray(x[b], np.float32).T),
            "wqT": np.ascontiguousarray(np.asarray(wq, np.float32)[sl].T),
            "wkT": np.ascontiguousarray(np.asarray(wk, np.float32)[sl].T),
            "wvT": np.ascontiguousarray(np.asarray(wv, np.float32)[sl].T),
            "wpT": np.ascontiguousarray(np.asarray(wproj, np.float32).T[sl]),
            "cos3": cos3,
            "sin3": sin3,
            "ident": ident,
        })
    return maps


def kernel(x, cos, sin, wq, wk, wv, wproj, _trace=False):
    nc = _get_nc()
    maps = _in_maps(x, cos, sin, wq, wk, wv, wproj)
    res = run_bass_kernel_spmd(nc, maps, core_ids=list(range(8)), trace=_trace)
    parts = [r["out"] for r in res.results]
    outv = np.stack([parts[2 * b] + parts[2 * b + 1] for b in range(B)]).astype(np.float32)
    if _trace:
        _CACHE["last_results"] = res
    return outv


# revision 13
# speedup vs baseline: 1.7689x; 1.7689x over previous
"""Causal self-attention (B=4, T=2048, C=768, H=6, D=128) on 8 trn2 NeuronCores.

Sharding: 24 (batch, head) units -> 8 cores, each core owns 1 batch x 3 heads.
Per core: QKV projections for its 3 heads, RoPE + per-head norm, causal
attention, partial output projection over its heads' columns.
Unshard: out[b] = partial[core 2b] + partial[core 2b+1]  (tensor-parallel sum).

Device-side layout choices:
  - Q,K computed in [t, d] layout (rope/norm are free-dim ops), then
    PE-transposed to [d, t] so the scores matmul contracts d on partitions.
  - scores computed TRANSPOSED: sT[s, q] = K[s]:Q[q] so that the AV matmul
    (contraction over s) can consume exp(sT) directly with V in natural [s, d]
    layout; output y arrives as yT[d, q], which is exactly the lhsT layout the
    output projection needs.  No attention-matrix transposes anywhere.
  - softmax denominator: exp tiles accumulated on DVE, partition-summed with a
    ones-vector matmul on PE, reciprocal on DVE, partition-broadcast on GPSIMD.
  - no max-subtraction in softmax: q,k are unit-normalized so |score| <= 11.4
    and exp stays comfortably inside fp32 range (matches reference softmax
    bit-for-bit up to rounding).
  - causal mask inside diagonal blocks: affine_select (iota predicate) zeroes
    exp values where s > q; fully-masked-out blocks are simply never computed.
"""

import numpy as np

import concourse.bacc as bacc
import concourse.bass as bass
import concourse.mybir as mybir
from concourse import tile
from concourse.bass_utils import run_bass_kernel_spmd

F32 = mybir.dt.float32
F32R = mybir.dt.float32r
AF = mybir.ActivationFunctionType
ALU = mybir.AluOpType


# fp32 matmuls run at 4 cycles/row on the PE; float32r (same 4-byte payload,
# different streaming mode) runs at 1 cycle/row for free dim >= 256, so every
# matmul operand tensor below is declared float32r end-to-end.

B, T, C, H, D = 4, 2048, 768, 6, 128
HALF = D // 2
NH = 3            # heads per core
CT = C // 128     # 6 contraction tiles for projections
NT = T // 128     # 16 token tiles
QC = 512          # query-chunk width for attention
NQC = T // QC     # 4 chunks
SCALE = 1.0 / float(np.sqrt(D))
EPS = 1e-6

_CACHE = {}


def _build_nc():
    nc = bacc.Bacc("TRN2")

    xT = nc.dram_tensor("xT", [C, T], F32R, kind="ExternalInput")
    wqT = nc.dram_tensor("wqT", [C, NH * D], F32R, kind="ExternalInput")
    wkT = nc.dram_tensor("wkT", [C, NH * D], F32R, kind="ExternalInput")
    wvT = nc.dram_tensor("wvT", [C, NH * D], F32R, kind="ExternalInput")
    wpT = nc.dram_tensor("wpT", [NH * D, C], F32R, kind="ExternalInput")
    cos3 = nc.dram_tensor("cos3", [T, NH * HALF], F32, kind="ExternalInput")
    sin3 = nc.dram_tensor("sin3", [T, NH * HALF], F32, kind="ExternalInput")
    ident = nc.dram_tensor("ident", [128, 128], F32, kind="ExternalInput")
    ones_in = nc.dram_tensor("ones_in", [128, 1], F32R, kind="ExternalInput")
    out = nc.dram_tensor("out", [T, C], F32, kind="ExternalOutput")

    with tile.TileContext(nc) as tc:
        with (
            tc.tile_pool(name="persist", bufs=1) as persist,
            tc.tile_pool(name="qkvbuf", bufs=1) as qkvbuf,
            tc.tile_pool(name="psA", bufs=4, space="PSUM") as psA,
            tc.tile_pool(name="psY", bufs=2, space="PSUM") as psY,
            tc.tile_pool(name="psT", bufs=2, space="PSUM") as psT,
        ):
            QT = qkvbuf.tile([128, NH, T], F32R)       # [d, h, t]
            KT = qkvbuf.tile([128, NH, T], F32R)       # [d, h, t]
            V = qkvbuf.tile([128, NT, NH * D], F32R)   # [s%128, s//128, h*D+d]
            ones = persist.tile([128, 1], F32R)
            nc.sync.dma_start(ones[:], ones_in[:])
            idn = persist.tile([128, 128], F32)
            nc.sync.dma_start(idn[:], ident[:])
            wp_sb = persist.tile([128, NH, C], F32R)   # [d, h, c]
            nc.sync.dma_start(wp_sb[:], wpT.rearrange("(h p) c -> p h c", p=128))

            # ---------------- stage 1+2: QKV projection + rope + norm ---------
            with (
                tc.tile_pool(name="wbuf", bufs=1) as wbuf,
                tc.tile_pool(name="xch", bufs=3) as xpool,
                tc.tile_pool(name="rope", bufs=3) as rpool,
                tc.tile_pool(name="stat", bufs=6) as spool,
            ):
                wq_sb = wbuf.tile([128, CT, NH * D], F32R)
                wk_sb = wbuf.tile([128, CT, NH * D], F32R)
                wv_sb = wbuf.tile([128, CT, NH * D], F32R)
                nc.sync.dma_start(wq_sb[:], wqT.rearrange("(ci p) o -> p ci o", p=128))
                nc.sync.dma_start(wk_sb[:], wkT.rearrange("(ci p) o -> p ci o", p=128))
                nc.sync.dma_start(wv_sb[:], wvT.rearrange("(ci p) o -> p ci o", p=128))
                cos_sb = wbuf.tile([128, NT, NH * HALF], F32)
                sin_sb = wbuf.tile([128, NT, NH * HALF], F32)
                nc.sync.dma_start(cos_sb[:], cos3.rearrange("(tt p) f -> p tt f", p=128))
                nc.sync.dma_start(sin_sb[:], sin3.rearrange("(tt p) f -> p tt f", p=128))

                xT_r = xT.rearrange("(ci p) (tt t) -> p ci tt t", p=128, t=128)

                for tt in range(NT):
                    xch = xpool.tile([128, CT, 128], F32R, tag="xch")
                    nc.sync.dma_start(xch[:], xT_r[:, :, tt, :])

                    qps = psA.tile([128, NH * D], F32, tag="ps")
                    kps = psA.tile([128, NH * D], F32, tag="ps")
                    vps = psA.tile([128, NH * D], F32, tag="ps")
                    for ci in range(CT):
                        st_, sp_ = (ci == 0), (ci == CT - 1)
                        lhs = xch[:, ci, :]
                        nc.tensor.matmul(qps[:], lhs, wq_sb[:, ci, :], start=st_, stop=sp_)
                        nc.tensor.matmul(kps[:], lhs, wk_sb[:, ci, :], start=st_, stop=sp_)
                        nc.tensor.matmul(vps[:], lhs, wv_sb[:, ci, :], start=st_, stop=sp_)

                    # V: straight copy PSUM -> SBUF in natural [t, o] layout
                    nc.scalar.copy(V[:, tt, :], vps[:])

                    cos_t = cos_sb[:, tt].rearrange("p (h f) -> p h f", h=NH)
                    sin_t = sin_sb[:, tt].rearrange("p (h f) -> p h f", h=NH)

                    # q/k stats share [128, 6] tiles (cols 0-2 q, 3-5 k) so the
                    # tiny per-stat ops run once per token tile, not twice.
                    sums = spool.tile([128, 2 * NH], F32, tag="sums")
                    sumsq = spool.tile([128, 2 * NH], F32, tag="sumsq")
                    rr = []
                    for mi, ps in enumerate((qps, kps)):
                        ps_v = ps[:].rearrange("p (h d) -> p h d", h=NH)
                        a = ps_v[:, :, 0:HALF]      # x1  [128, 3, 64]
                        b = ps_v[:, :, HALF:D]      # x2  [128, 3, 64]
                        r = rpool.tile([128, NH * D], F32, tag=f"r{mi}")
                        rr.append(r)
                        r_v = r[:].rearrange("p (h d) -> p h d", h=NH)
                        r1 = r_v[:, :, 0:HALF]
                        r2 = r_v[:, :, HALF:D]
                        tbs = rpool.tile([128, NH * HALF], F32, tag="tbs")
                        tas = rpool.tile([128, NH * HALF], F32, tag="tas")
                        tbs_v = tbs[:].rearrange("p (h f) -> p h f", h=NH)
                        tas_v = tas[:].rearrange("p (h f) -> p h f", h=NH)
                        # rope: r1 = a*cos + b*sin ; r2 = b*cos - a*sin
                        nc.vector.tensor_mul(tbs_v, b, sin_t)
                        nc.vector.tensor_mul(tas_v, a, sin_t)
                        nc.vector.tensor_mul(r1, a, cos_t)
                        nc.vector.tensor_mul(r2, b, cos_t)
                        nc.vector.tensor_add(r1, r1, tbs_v)
                        nc.vector.tensor_sub(r2, r2, tas_v)

                        sl = slice(mi * NH, (mi + 1) * NH)
                        nc.vector.tensor_reduce(sums[:, sl], r_v, axis=mybir.AxisListType.X, op=ALU.add)
                        sq = rpool.tile([128, NH * D], F32, tag="sq")
                        for h in range(NH):
                            nc.scalar.activation(
                                sq[:, h * D:(h + 1) * D], r[:, h * D:(h + 1) * D],
                                AF.Square, accum_out=sumsq[:, mi * NH + h:mi * NH + h + 1],
                            )
                    # rstd = 1/(sqrt((sumsq - mean*sum)/127) + eps), ddof=1
                    negmean = spool.tile([128, 2 * NH], F32, tag="negmean")
                    nc.scalar.mul(negmean[:], sums[:], -1.0 / D)
                    var = spool.tile([128, 2 * NH], F32, tag="var")
                    nc.vector.tensor_mul(var[:], negmean[:], sums[:])
                    nc.vector.tensor_add(var[:], sumsq[:], var[:])
                    stdv = spool.tile([128, 2 * NH], F32, tag="stdv")
                    nc.scalar.activation(stdv[:], var[:], AF.Sqrt, scale=1.0 / (D - 1))
                    nc.vector.tensor_scalar_add(stdv[:], stdv[:], EPS)
                    rstd = spool.tile([128, 2 * NH], F32, tag="rstd")
                    nc.vector.reciprocal(rstd[:], stdv[:])
                    nmrs = spool.tile([128, 2 * NH], F32, tag="nmrs")
                    nc.vector.tensor_mul(nmrs[:], negmean[:], rstd[:])

                    for mi, dstT in ((0, QT), (1, KT)):
                        r = rr[mi]
                        nrm = rpool.tile([128, NH * D], F32, tag="nrm")
                        tps = psT.tile([128, QC], F32, tag="tp")
                        for h in range(NH):
                            c = mi * NH + h
                            # (r - mean)*rstd = r*rstd + (-mean*rstd)
                            nc.scalar.activation(
                                nrm[:, h * D:(h + 1) * D], r[:, h * D:(h + 1) * D],
                                AF.Identity, bias=nmrs[:, c:c + 1], scale=rstd[:, c:c + 1],
                            )
                            nc.tensor.transpose(tps[:, h * D:(h + 1) * D],
                                                nrm[:, h * D:(h + 1) * D], idn[:])
                        # one strided copy moves all 3 transposed heads out
                        dst = dstT[:, :, tt * 128:(tt + 1) * 128]
                        src = tps[:, 0:NH * D].rearrange("p (h t) -> p h t", h=NH)
                        nc.vector.tensor_copy(dst, src)

            # ---------------- stage 3+4: attention + output projection --------
            with (
                tc.tile_pool(name="att", bufs=3) as apool,
                tc.tile_pool(name="acc", bufs=2) as accpool,
                tc.tile_pool(name="ybuf", bufs=2) as ypool,
                tc.tile_pool(name="obuf", bufs=3) as opool,
            ):
                out_r = out.rearrange("(tt p) c -> p tt c", p=128)
                for qc in range(NQC):
                    Q0 = qc * QC
                    n_st = (Q0 + QC) // 128
                    yTc = ypool.tile([128, NH, QC], F32R, tag="yT")  # [d, h, q]
                    for h in range(NH):
                        yps = psY.tile([128, QC], F32, tag="yps")
                        dps = psT.tile([128, QC], F32, tag="tp")
                        for st in range(n_st):
                            sps = psA.tile([128, QC], F32, tag="ps")
                            nc.tensor.matmul(
                                sps[:],
                                KT[:, h, st * 128:(st + 1) * 128],
                                QT[:, h, Q0:Q0 + QC],
                                start=True, stop=True,
                            )
                            et = apool.tile([128, QC], F32R, tag="et")
                            nc.scalar.activation(et[:], sps[:], AF.Exp, scale=SCALE)
                            if st * 128 >= Q0:  # diagonal block: zero where s > q
                                k = st - n_st + 4
                                nc.gpsimd.affine_select(
                                    et[:], et[:],
                                    pattern=[[1, QC]],
                                    compare_op=ALU.is_ge,
                                    fill=0.0,
                                    base=-(128 * k),
                                    channel_multiplier=-1,
                                )
                            nc.tensor.matmul(
                                yps[:],
                                V[:, st, h * D:(h + 1) * D],
                                et[:],
                                start=(st == 0), stop=(st == n_st - 1),
                                skip_group_check=True,
                            )
                            # softmax denominator on PE: accumulate ones^T @ exp
                            nc.tensor.matmul(
                                dps[:1, :],
                                ones[:],
                                et[:],
                                start=(st == 0), stop=(st == n_st - 1),
                                skip_group_check=True,
                            )
                        d1 = accpool.tile([128, QC], F32, tag="d1")
                        nc.scalar.copy(d1[:1, :], dps[:1, :])
                        dbc = accpool.tile([128, QC], F32, tag="dbc")
                        nc.gpsimd.partition_broadcast(dbc[:], d1[:1, :])
                        rbc = accpool.tile([128, QC], F32, tag="rbc")
                        nc.vector.reciprocal(rbc[:], dbc[:])
                        nc.vector.tensor_mul(yTc[:, h, :], yps[:], rbc[:])

                    # output projection for this chunk's 4 token tiles
                    for j in range(QC // 128):
                        tt = qc * (QC // 128) + j
                        op0 = psA.tile([128, C // 2], F32, tag="ps")
                        op1 = psA.tile([128, C // 2], F32, tag="ps")
                        for h in range(NH):
                            lhs = yTc[:, h, j * 128:(j + 1) * 128]
                            nc.tensor.matmul(op0[:], lhs, wp_sb[:, h, 0:C // 2],
                                             start=(h == 0), stop=(h == NH - 1))
                            nc.tensor.matmul(op1[:], lhs, wp_sb[:, h, C // 2:C],
                                             start=(h == 0), stop=(h == NH - 1))
                        ot = opool.tile([128, C], F32, tag="ot")
                        nc.scalar.copy(ot[:, 0:C // 2], op0[:])
                        nc.scalar.copy(ot[:, C // 2:C], op1[:])
                        nc.sync.dma_start(out_r[:, tt, :], ot[:])

    nc.compile()
    return nc


def _get_nc():
    if "nc" not in _CACHE:
        _CACHE["nc"] = _build_nc()
    return _CACHE["nc"]


def _in_maps(x, cos, sin, wq, wk, wv, wproj):
    cos3 = np.ascontiguousarray(np.tile(np.asarray(cos, np.float32), (1, NH)))
    sin3 = np.ascontiguousarray(np.tile(np.asarray(sin, np.float32), (1, NH)))
    ident = np.eye(128, dtype=np.float32)
    maps = []
    for c in range(8):
        b = c // 2
        hs = (c % 2) * NH
        sl = slice(hs * D, (hs + NH) * D)
        maps.append({
            "xT": np.ascontiguousarray(np.asarray(x[b], np.float32).T),
            "wqT": np.ascontiguousarray(np.asarray(wq, np.float32)[sl].T),
            "wkT": np.ascontiguousarray(np.asarray(wk, np.float32)[sl].T),
            "wvT": np.ascontiguousarray(np.asarray(wv, np.float32)[sl].T),
            "wpT": np.ascontiguousarray(np.asarray(wproj, np.float32).T[sl]),
            "cos3": cos3,
            "sin3": sin3,
            "ident": ident,
            "ones_in": np.ones((128, 1), dtype=np.float32),
        })
    return maps


def kernel(x, cos, sin, wq, wk, wv, wproj, _trace=False):
    nc = _get_nc()
    maps = _in_maps(x, cos, sin, wq, wk, wv, wproj)
    res = run_bass_kernel_spmd(nc, maps, core_ids=list(range(8)), trace=_trace)
    parts = [r["out"] for r in res.results]
    outv = np.stack([parts[2 * b] + parts[2 * b + 1] for b in range(B)]).astype(np.float32)
    if _trace:
        _CACHE["last_results"] = res
    return outv


# revision 19
# speedup vs baseline: 1.7778x; 1.0050x over previous
"""Causal self-attention (B=4, T=2048, C=768, H=6, D=128) on 8 trn2 NeuronCores.

Sharding: 24 (batch, head) units -> 8 cores, each core owns 1 batch x 3 heads.
Per core: QKV projections for its 3 heads, RoPE + per-head norm, causal
attention, partial output projection over its heads' columns.
Unshard: out[b] = partial[core 2b] + partial[core 2b+1]  (tensor-parallel sum).

Device-side layout choices:
  - Q,K computed in [t, d] layout (rope/norm are free-dim ops), then
    PE-transposed to [d, t] so the scores matmul contracts d on partitions.
  - scores computed TRANSPOSED: sT[s, q] = K[s]:Q[q] so that the AV matmul
    (contraction over s) can consume exp(sT) directly with V in natural [s, d]
    layout; output y arrives as yT[d, q], which is exactly the lhsT layout the
    output projection needs.  No attention-matrix transposes anywhere.
  - softmax denominator: exp tiles accumulated on DVE, partition-summed with a
    ones-vector matmul on PE, reciprocal on DVE, partition-broadcast on GPSIMD.
  - no max-subtraction in softmax: q,k are unit-normalized so |score| <= 11.4
    and exp stays comfortably inside fp32 range (matches reference softmax
    bit-for-bit up to rounding).
  - causal mask inside diagonal blocks: affine_select (iota predicate) zeroes
    exp values where s > q; fully-masked-out blocks are simply never computed.
"""

import numpy as np

import concourse.bacc as bacc
import concourse.bass as bass
import concourse.mybir as mybir
from concourse import tile
from concourse.bass_utils import run_bass_kernel_spmd

F32 = mybir.dt.float32
F32R = mybir.dt.float32r
AF = mybir.ActivationFunctionType
ALU = mybir.AluOpType


# fp32 matmuls run at 4 cycles/row on the PE; float32r (same 4-byte payload,
# different streaming mode) runs at 1 cycle/row for free dim >= 256, so every
# matmul operand tensor below is declared float32r end-to-end.

B, T, C, H, D = 4, 2048, 768, 6, 128
HALF = D // 2
NH = 3            # heads per core
CT = C // 128     # 6 contraction tiles for projections
NT = T // 128     # 16 token tiles
QC = 512          # query-chunk width for attention
NQC = T // QC     # 4 chunks
SCALE = 1.0 / float(np.sqrt(D))
EPS = 1e-6

_CACHE = {}


def _build_nc():
    nc = bacc.Bacc("TRN2")

    xT = nc.dram_tensor("xT", [C, T], F32R, kind="ExternalInput")
    wqT = nc.dram_tensor("wqT", [C, NH * D], F32R, kind="ExternalInput")
    wkT = nc.dram_tensor("wkT", [C, NH * D], F32R, kind="ExternalInput")
    wvT = nc.dram_tensor("wvT", [C, NH * D], F32R, kind="ExternalInput")
    wpT = nc.dram_tensor("wpT", [NH * D, C], F32R, kind="ExternalInput")
    cos3 = nc.dram_tensor("cos3", [T, NH * HALF], F32, kind="ExternalInput")
    sin3 = nc.dram_tensor("sin3", [T, NH * HALF], F32, kind="ExternalInput")
    ident = nc.dram_tensor("ident", [128, 128], F32, kind="ExternalInput")
    ones_in = nc.dram_tensor("ones_in", [128, 1], F32R, kind="ExternalInput")
    out = nc.dram_tensor("out", [T, C], F32, kind="ExternalOutput")

    with tile.TileContext(nc) as tc:
        with (
            tc.tile_pool(name="persist", bufs=1) as persist,
            tc.tile_pool(name="qkvbuf", bufs=1) as qkvbuf,
            tc.tile_pool(name="psA", bufs=3, space="PSUM") as psA,
            tc.tile_pool(name="psY", bufs=3, space="PSUM") as psY,
            tc.tile_pool(name="psT", bufs=2, space="PSUM") as psT,
        ):
            QT = qkvbuf.tile([128, NH, T], F32R)       # [d, h, t]
            KT = qkvbuf.tile([128, NH, T], F32R)       # [d, h, t]
            V = qkvbuf.tile([128, NT, NH * D], F32R)   # [s%128, s//128, h*D+d]
            ones = persist.tile([128, 1], F32R)
            idn = persist.tile([128, 128], F32)
            wp_sb = persist.tile([128, NH, C], F32R)   # [d, h, c]

            # ---------------- stage 1+2: QKV projection + rope + norm ---------
            with (
                tc.tile_pool(name="wbuf", bufs=1) as wbuf,
                tc.tile_pool(name="xch", bufs=3) as xpool,
                tc.tile_pool(name="rope", bufs=3) as rpool,
                tc.tile_pool(name="stat", bufs=6) as spool,
            ):
                wq_sb = wbuf.tile([128, CT, NH * D], F32R)
                wk_sb = wbuf.tile([128, CT, NH * D], F32R)
                wv_sb = wbuf.tile([128, CT, NH * D], F32R)
                # startup-latency ordering: first-tile deps (weights, x tile 0)
                # are issued first; cos/sin next (needed ~us later); wp/ident/
                # ones last (needed only after the first transpose / in stage 3)
                nc.sync.dma_start(wq_sb[:], wqT.rearrange("(ci p) o -> p ci o", p=128))
                nc.sync.dma_start(wk_sb[:], wkT.rearrange("(ci p) o -> p ci o", p=128))
                nc.sync.dma_start(wv_sb[:], wvT.rearrange("(ci p) o -> p ci o", p=128))

                xT_r = xT.rearrange("(ci p) (tt t) -> p ci tt t", p=128, t=128)
                xch0 = xpool.tile([128, CT, 128], F32R, tag="xch")
                nc.sync.dma_start(xch0[:], xT_r[:, :, 0, :])

                cos_sb = wbuf.tile([128, NT, NH * HALF], F32)
                sin_sb = wbuf.tile([128, NT, NH * HALF], F32)
                nc.sync.dma_start(cos_sb[:], cos3.rearrange("(tt p) f -> p tt f", p=128))
                nc.sync.dma_start(sin_sb[:], sin3.rearrange("(tt p) f -> p tt f", p=128))
                nc.sync.dma_start(idn[:], ident[:])
                nc.sync.dma_start(wp_sb[:], wpT.rearrange("(h p) c -> p h c", p=128))
                nc.sync.dma_start(ones[:], ones_in[:])

                for tt in range(NT):
                    if tt == 0:
                        xch = xch0
                    else:
                        xch = xpool.tile([128, CT, 128], F32R, tag="xch")
                        nc.sync.dma_start(xch[:], xT_r[:, :, tt, :])

                    qps = psA.tile([128, NH * D], F32, tag="ps")
                    kps = psA.tile([128, NH * D], F32, tag="ps")
                    vps = psA.tile([128, NH * D], F32, tag="ps")
                    for ci in range(CT):
                        st_, sp_ = (ci == 0), (ci == CT - 1)
                        lhs = xch[:, ci, :]
                        nc.tensor.matmul(qps[:], lhs, wq_sb[:, ci, :], start=st_, stop=sp_)
                        nc.tensor.matmul(kps[:], lhs, wk_sb[:, ci, :], start=st_, stop=sp_)
                        nc.tensor.matmul(vps[:], lhs, wv_sb[:, ci, :], start=st_, stop=sp_)

                    # V: straight copy PSUM -> SBUF in natural [t, o] layout
                    nc.scalar.copy(V[:, tt, :], vps[:])

                    cos_t = cos_sb[:, tt].rearrange("p (h f) -> p h f", h=NH)
                    sin_t = sin_sb[:, tt].rearrange("p (h f) -> p h f", h=NH)

                    # q/k stats share [128, 6] tiles (cols 0-2 q, 3-5 k) so the
                    # tiny per-stat ops run once per token tile, not twice.
                    sums = spool.tile([128, 2 * NH], F32, tag="sums")
                    sumsq = spool.tile([128, 2 * NH], F32, tag="sumsq")
                    rr = []
                    for mi, ps in enumerate((qps, kps)):
                        ps_v = ps[:].rearrange("p (h d) -> p h d", h=NH)
                        a = ps_v[:, :, 0:HALF]      # x1  [128, 3, 64]
                        b = ps_v[:, :, HALF:D]      # x2  [128, 3, 64]
                        r = rpool.tile([128, NH * D], F32, tag=f"r{mi}")
                        rr.append(r)
                        r_v = r[:].rearrange("p (h d) -> p h d", h=NH)
                        r1 = r_v[:, :, 0:HALF]
                        r2 = r_v[:, :, HALF:D]
                        tbs = rpool.tile([128, NH * HALF], F32, tag="tbs")
                        tas = rpool.tile([128, NH * HALF], F32, tag="tas")
                        tbs_v = tbs[:].rearrange("p (h f) -> p h f", h=NH)
                        tas_v = tas[:].rearrange("p (h f) -> p h f", h=NH)
                        # rope: r1 = a*cos + b*sin ; r2 = b*cos - a*sin
                        nc.vector.tensor_mul(tbs_v, b, sin_t)
                        nc.vector.tensor_mul(tas_v, a, sin_t)
                        nc.vector.tensor_mul(r1, a, cos_t)
                        nc.vector.tensor_mul(r2, b, cos_t)
                        nc.vector.tensor_add(r1, r1, tbs_v)
                        nc.vector.tensor_sub(r2, r2, tas_v)

                        sl = slice(mi * NH, (mi + 1) * NH)
                        nc.vector.tensor_reduce(sums[:, sl], r_v, axis=mybir.AxisListType.X, op=ALU.add)
                        sq = rpool.tile([128, NH * D], F32, tag="sq")
                        for h in range(NH):
                            nc.scalar.activation(
                                sq[:, h * D:(h + 1) * D], r[:, h * D:(h + 1) * D],
                                AF.Square, accum_out=sumsq[:, mi * NH + h:mi * NH + h + 1],
                            )
                    # rstd = 1/(sqrt((sumsq - mean*sum)/127) + eps), ddof=1
                    negmean = spool.tile([128, 2 * NH], F32, tag="negmean")
                    nc.scalar.mul(negmean[:], sums[:], -1.0 / D)
                    var = spool.tile([128, 2 * NH], F32, tag="var")
                    nc.vector.tensor_mul(var[:], negmean[:], sums[:])
                    nc.vector.tensor_add(var[:], sumsq[:], var[:])
                    stdv = spool.tile([128, 2 * NH], F32, tag="stdv")
                    nc.scalar.activation(stdv[:], var[:], AF.Sqrt, scale=1.0 / (D - 1))
                    nc.vector.tensor_scalar_add(stdv[:], stdv[:], EPS)
                    rstd = spool.tile([128, 2 * NH], F32, tag="rstd")
                    nc.vector.reciprocal(rstd[:], stdv[:])
                    nmrs = spool.tile([128, 2 * NH], F32, tag="nmrs")
                    nc.vector.tensor_mul(nmrs[:], negmean[:], rstd[:])

                    for mi, dstT in ((0, QT), (1, KT)):
                        r = rr[mi]
                        nrm = rpool.tile([128, NH * D], F32, tag="nrm")
                        tps = psT.tile([128, QC], F32, tag="tp")
                        for h in range(NH):
                            c = mi * NH + h
                            # (r - mean)*rstd = r*rstd + (-mean*rstd)
                            nc.scalar.activation(
                                nrm[:, h * D:(h + 1) * D], r[:, h * D:(h + 1) * D],
                                AF.Identity, bias=nmrs[:, c:c + 1], scale=rstd[:, c:c + 1],
                            )
                            nc.tensor.transpose(tps[:, h * D:(h + 1) * D],
                                                nrm[:, h * D:(h + 1) * D], idn[:])
                        # one strided copy moves all 3 transposed heads out
                        dst = dstT[:, :, tt * 128:(tt + 1) * 128]
                        src = tps[:, 0:NH * D].rearrange("p (h t) -> p h t", h=NH)
                        nc.vector.tensor_copy(dst, src)

            # ---------------- stage 3+4: attention + output projection --------
            with (
                tc.tile_pool(name="att", bufs=3) as apool,
                tc.tile_pool(name="acc", bufs=2) as accpool,
                tc.tile_pool(name="ybuf", bufs=2) as ypool,
                tc.tile_pool(name="obuf", bufs=3) as opool,
            ):
                out_r = out.rearrange("(tt p) c -> p tt c", p=128)
                for qc in range(NQC):
                    Q0 = qc * QC
                    n_st = (Q0 + QC) // 128
                    yTc = ypool.tile([128, NH, QC], F32R, tag="yT")  # [d, h, q]
                    for h in range(NH):
                        yps = psY.tile([128, QC], F32, tag="yps")
                        dps = psT.tile([128, QC], F32, tag="tp")
                        for st in range(n_st):
                            sps = psA.tile([128, QC], F32, tag="ps")
                            nc.tensor.matmul(
                                sps[:],
                                KT[:, h, st * 128:(st + 1) * 128],
                                QT[:, h, Q0:Q0 + QC],
                                start=True, stop=True,
                            )
                            et = apool.tile([128, QC], F32R, tag="et")
                            nc.scalar.activation(et[:], sps[:], AF.Exp, scale=SCALE)
                            if st * 128 >= Q0:  # diagonal block: zero where s > q
                                k = st - n_st + 4
                                nc.gpsimd.affine_select(
                                    et[:], et[:],
                                    pattern=[[1, QC]],
                                    compare_op=ALU.is_ge,
                                    fill=0.0,
                                    base=-(128 * k),
                                    channel_multiplier=-1,
                                )
                            nc.tensor.matmul(
                                yps[:],
                                V[:, st, h * D:(h + 1) * D],
                                et[:],
                                start=(st == 0), stop=(st == n_st - 1),
                                skip_group_check=True,
                            )
                            # softmax denominator on PE: accumulate ones^T @ exp
                            nc.tensor.matmul(
                                dps[:1, :],
                                ones[:],
                                et[:],
                                start=(st == 0), stop=(st == n_st - 1),
                                skip_group_check=True,
                            )
                        rc1 = accpool.tile([128, QC], F32, tag="rc1")
                        nc.vector.reciprocal(rc1[:1, :], dps[:1, :])
                        rbc = accpool.tile([128, QC], F32, tag="rbc")
                        nc.gpsimd.partition_broadcast(rbc[:], rc1[:1, :])
                        nc.vector.tensor_mul(yTc[:, h, :], yps[:], rbc[:])

                    # output projection for this chunk's 4 token tiles
                    for j in range(QC // 128):
                        tt = qc * (QC // 128) + j
                        op0 = psA.tile([128, C // 2], F32, tag="ps")
                        op1 = psA.tile([128, C // 2], F32, tag="ps")
                        for h in range(NH):
                            lhs = yTc[:, h, j * 128:(j + 1) * 128]
                            nc.tensor.matmul(op0[:], lhs, wp_sb[:, h, 0:C // 2],
                                             start=(h == 0), stop=(h == NH - 1))
                            nc.tensor.matmul(op1[:], lhs, wp_sb[:, h, C // 2:C],
                                             start=(h == 0), stop=(h == NH - 1))
                        ot = opool.tile([128, C], F32, tag="ot")
                        nc.scalar.copy(ot[:, 0:C // 2], op0[:])
                        nc.scalar.copy(ot[:, C // 2:C], op1[:])
                        nc.sync.dma_start(out_r[:, tt, :], ot[:])

    nc.compile()
    return nc


def _get_nc():
    if "nc" not in _CACHE:
        _CACHE["nc"] = _build_nc()
    return _CACHE["nc"]


def _in_maps(x, cos, sin, wq, wk, wv, wproj):
    cos3 = np.ascontiguousarray(np.tile(np.asarray(cos, np.float32), (1, NH)))
    sin3 = np.ascontiguousarray(np.tile(np.asarray(sin, np.float32), (1, NH)))
    ident = np.eye(128, dtype=np.float32)
    maps = []
    for c in range(8):
        b = c // 2
        hs = (c % 2) * NH
        sl = slice(hs * D, (hs + NH) * D)
        maps.append({
            "xT": np.ascontiguousarray(np.asarray(x[b], np.float32).T),
            "wqT": np.ascontiguousarray(np.asarray(wq, np.float32)[sl].T),
            "wkT": np.ascontiguousarray(np.asarray(wk, np.float32)[sl].T),
            "wvT": np.ascontiguousarray(np.asarray(wv, np.float32)[sl].T),
            "wpT": np.ascontiguousarray(np.asarray(wproj, np.float32).T[sl]),
            "cos3": cos3,
            "sin3": sin3,
            "ident": ident,
            "ones_in": np.ones((128, 1), dtype=np.float32),
        })
    return maps


def kernel(x, cos, sin, wq, wk, wv, wproj, _trace=False):
    nc = _get_nc()
    maps = _in_maps(x, cos, sin, wq, wk, wv, wproj)
    res = run_bass_kernel_spmd(nc, maps, core_ids=list(range(8)), trace=_trace)
    parts = [r["out"] for r in res.results]
    outv = np.stack([parts[2 * b] + parts[2 * b + 1] for b in range(B)]).astype(np.float32)
    if _trace:
        _CACHE["last_results"] = res
    return outv


# revision 20
# speedup vs baseline: 1.9282x; 1.0846x over previous
"""Causal self-attention (B=4, T=2048, C=768, H=6, D=128) on 8 trn2 NeuronCores.

Sharding: 24 (batch, head) units -> 8 cores, each core owns 1 batch x 3 heads.
Per core: QKV projections for its 3 heads, RoPE + per-head norm, causal
attention, partial output projection over its heads' columns.
Unshard: out[b] = partial[core 2b] + partial[core 2b+1]  (tensor-parallel sum).

Device-side layout choices:
  - Q,K computed in [t, d] layout (rope/norm are free-dim ops), then
    PE-transposed to [d, t] so the scores matmul contracts d on partitions.
  - scores computed TRANSPOSED: sT[s, q] = K[s]:Q[q] so that the AV matmul
    (contraction over s) can consume exp(sT) directly with V in natural [s, d]
    layout; output y arrives as yT[d, q], which is exactly the lhsT layout the
    output projection needs.  No attention-matrix transposes anywhere.
  - softmax denominator: exp tiles accumulated on DVE, partition-summed with a
    ones-vector matmul on PE, reciprocal on DVE, partition-broadcast on GPSIMD.
  - no max-subtraction in softmax: q,k are unit-normalized so |score| <= 11.4
    and exp stays comfortably inside fp32 range (matches reference softmax
    bit-for-bit up to rounding).
  - causal mask inside diagonal blocks: affine_select (iota predicate) zeroes
    exp values where s > q; fully-masked-out blocks are simply never computed.
"""

import numpy as np

import concourse.bacc as bacc
import concourse.bass as bass
import concourse.mybir as mybir
from concourse import tile
from concourse.bass_utils import run_bass_kernel_spmd

F32 = mybir.dt.float32
F32R = mybir.dt.float32r
AF = mybir.ActivationFunctionType
ALU = mybir.AluOpType


# fp32 matmuls run at 4 cycles/row on the PE; float32r (same 4-byte payload,
# different streaming mode) runs at 1 cycle/row for free dim >= 256, so every
# matmul operand tensor below is declared float32r end-to-end.

B, T, C, H, D = 4, 2048, 768, 6, 128
HALF = D // 2
NH = 3            # heads per core
CT = C // 128     # 6 contraction tiles for projections
NT = T // 128     # 16 token tiles
QC = 512          # query-chunk width for attention
NQC = T // QC     # 4 chunks
SCALE = 1.0 / float(np.sqrt(D))
EPS = 1e-6

_CACHE = {}


def _build_nc():
    nc = bacc.Bacc("TRN2")

    xT = nc.dram_tensor("xT", [C, T], F32R, kind="ExternalInput")
    wqT = nc.dram_tensor("wqT", [C, NH * D], F32R, kind="ExternalInput")
    wkT = nc.dram_tensor("wkT", [C, NH * D], F32R, kind="ExternalInput")
    wvT = nc.dram_tensor("wvT", [C, NH * D], F32R, kind="ExternalInput")
    wpT = nc.dram_tensor("wpT", [NH * D, C], F32R, kind="ExternalInput")
    cos3 = nc.dram_tensor("cos3", [T, NH * HALF], F32, kind="ExternalInput")
    sin3 = nc.dram_tensor("sin3", [T, NH * HALF], F32, kind="ExternalInput")
    ident = nc.dram_tensor("ident", [128, 128], F32, kind="ExternalInput")
    ones_in = nc.dram_tensor("ones_in", [128, 1], F32R, kind="ExternalInput")
    out = nc.dram_tensor("out", [T, C], F32, kind="ExternalOutput")

    with tile.TileContext(nc) as tc:
        with (
            tc.tile_pool(name="persist", bufs=1) as persist,
            tc.tile_pool(name="qkvbuf", bufs=1) as qkvbuf,
            tc.tile_pool(name="psA", bufs=3, space="PSUM") as psA,
            tc.tile_pool(name="psY", bufs=3, space="PSUM") as psY,
            tc.tile_pool(name="psT", bufs=2, space="PSUM") as psT,
        ):
            QT = qkvbuf.tile([128, NH, T], F32R)       # [d, h, t]
            KT = qkvbuf.tile([128, NH, T], F32R)       # [d, h, t]
            V = qkvbuf.tile([128, NT, NH * D], F32R)   # [s%128, s//128, h*D+d]
            ones = persist.tile([128, 1], F32R)
            idn = persist.tile([128, 128], F32)
            wp_sb = persist.tile([128, NH, C], F32R)   # [d, h, c]

            # ---------------- stage 1+2: QKV projection + rope + norm ---------
            with (
                tc.tile_pool(name="wbuf", bufs=1) as wbuf,
                tc.tile_pool(name="xch", bufs=3) as xpool,
                tc.tile_pool(name="rope", bufs=3) as rpool,
                tc.tile_pool(name="stat", bufs=6) as spool,
            ):
                wq_sb = wbuf.tile([128, CT, NH * D], F32R)
                wk_sb = wbuf.tile([128, CT, NH * D], F32R)
                wv_sb = wbuf.tile([128, CT, NH * D], F32R)
                # startup-latency ordering: first-tile deps (weights, x tile 0)
                # are issued first; cos/sin next (needed ~us later); wp/ident/
                # ones last (needed only after the first transpose / in stage 3)
                nc.sync.dma_start(wq_sb[:], wqT.rearrange("(ci p) o -> p ci o", p=128))
                nc.sync.dma_start(wk_sb[:], wkT.rearrange("(ci p) o -> p ci o", p=128))
                nc.sync.dma_start(wv_sb[:], wvT.rearrange("(ci p) o -> p ci o", p=128))

                xT_r = xT.rearrange("(ci p) (tt t) -> p ci tt t", p=128, t=128)
                xch0 = xpool.tile([128, CT, 128], F32R, tag="xch")
                nc.sync.dma_start(xch0[:], xT_r[:, :, 0, :])

                cos_sb = wbuf.tile([128, NT, NH * HALF], F32)
                sin_sb = wbuf.tile([128, NT, NH * HALF], F32)
                nc.sync.dma_start(cos_sb[:], cos3.rearrange("(tt p) f -> p tt f", p=128))
                nc.sync.dma_start(sin_sb[:], sin3.rearrange("(tt p) f -> p tt f", p=128))
                nc.sync.dma_start(idn[:], ident[:])
                nc.sync.dma_start(wp_sb[:], wpT.rearrange("(h p) c -> p h c", p=128))
                nc.sync.dma_start(ones[:], ones_in[:])

                for tt in range(NT):
                    if tt == 0:
                        xch = xch0
                    else:
                        xch = xpool.tile([128, CT, 128], F32R, tag="xch")
                        nc.sync.dma_start(xch[:], xT_r[:, :, tt, :])

                    qps = psA.tile([128, NH * D], F32, tag="ps")
                    kps = psA.tile([128, NH * D], F32, tag="ps")
                    vps = psA.tile([128, NH * D], F32, tag="ps")
                    for ci in range(CT):
                        st_, sp_ = (ci == 0), (ci == CT - 1)
                        lhs = xch[:, ci, :]
                        nc.tensor.matmul(qps[:], lhs, wq_sb[:, ci, :], start=st_, stop=sp_)
                        nc.tensor.matmul(kps[:], lhs, wk_sb[:, ci, :], start=st_, stop=sp_)
                        nc.tensor.matmul(vps[:], lhs, wv_sb[:, ci, :], start=st_, stop=sp_)

                    # V: straight copy PSUM -> SBUF in natural [t, o] layout
                    nc.scalar.copy(V[:, tt, :], vps[:])

                    cos_t = cos_sb[:, tt].rearrange("p (h f) -> p h f", h=NH)
                    sin_t = sin_sb[:, tt].rearrange("p (h f) -> p h f", h=NH)

                    # q/k stats share [128, 6] tiles (cols 0-2 q, 3-5 k) so the
                    # tiny per-stat ops run once per token tile, not twice.
                    sums = spool.tile([128, 2 * NH], F32, tag="sums")
                    sumsq = spool.tile([128, 2 * NH], F32, tag="sumsq")
                    rr = []
                    for mi, ps in enumerate((qps, kps)):
                        ps_v = ps[:].rearrange("p (h d) -> p h d", h=NH)
                        a = ps_v[:, :, 0:HALF]      # x1  [128, 3, 64]
                        b = ps_v[:, :, HALF:D]      # x2  [128, 3, 64]
                        r = rpool.tile([128, NH * D], F32, tag=f"r{mi}")
                        rr.append(r)
                        r_v = r[:].rearrange("p (h d) -> p h d", h=NH)
                        r1 = r_v[:, :, 0:HALF]
                        r2 = r_v[:, :, HALF:D]
                        tbs = rpool.tile([128, NH * HALF], F32, tag="tbs")
                        tas = rpool.tile([128, NH * HALF], F32, tag="tas")
                        tbs_v = tbs[:].rearrange("p (h f) -> p h f", h=NH)
                        tas_v = tas[:].rearrange("p (h f) -> p h f", h=NH)
                        # rope: r1 = a*cos + b*sin ; r2 = b*cos - a*sin
                        nc.vector.tensor_mul(tbs_v, b, sin_t)
                        nc.vector.tensor_mul(tas_v, a, sin_t)
                        nc.vector.tensor_mul(r1, a, cos_t)
                        nc.vector.tensor_mul(r2, b, cos_t)
                        nc.vector.tensor_add(r1, r1, tbs_v)
                        nc.vector.tensor_sub(r2, r2, tas_v)

                        sl = slice(mi * NH, (mi + 1) * NH)
                        nc.vector.tensor_reduce(sums[:, sl], r_v, axis=mybir.AxisListType.X, op=ALU.add)
                        sq = rpool.tile([128, NH * D], F32, tag="sq")
                        for h in range(NH):
                            nc.scalar.activation(
                                sq[:, h * D:(h + 1) * D], r[:, h * D:(h + 1) * D],
                                AF.Square, accum_out=sumsq[:, mi * NH + h:mi * NH + h + 1],
                            )
                    # rstd = 1/(sqrt((sumsq - mean*sum)/127) + eps), ddof=1
                    negmean = spool.tile([128, 2 * NH], F32, tag="negmean")
                    nc.scalar.mul(negmean[:], sums[:], -1.0 / D)
                    var = spool.tile([128, 2 * NH], F32, tag="var")
                    nc.vector.tensor_mul(var[:], negmean[:], sums[:])
                    nc.vector.tensor_add(var[:], sumsq[:], var[:])
                    stdv = spool.tile([128, 2 * NH], F32, tag="stdv")
                    nc.scalar.activation(stdv[:], var[:], AF.Sqrt, scale=1.0 / (D - 1))
                    nc.vector.tensor_scalar_add(stdv[:], stdv[:], EPS)
                    rstd = spool.tile([128, 2 * NH], F32, tag="rstd")
                    nc.vector.reciprocal(rstd[:], stdv[:])
                    nmrs = spool.tile([128, 2 * NH], F32, tag="nmrs")
                    nc.vector.tensor_mul(nmrs[:], negmean[:], rstd[:])

                    for mi, dstT in ((0, QT), (1, KT)):
                        r = rr[mi]
                        nrm = rpool.tile([128, NH * D], F32, tag="nrm")
                        tps = psT.tile([128, QC], F32, tag="tp")
                        for h in range(NH):
                            c = mi * NH + h
                            # (r - mean)*rstd = r*rstd + (-mean*rstd)
                            nc.scalar.activation(
                                nrm[:, h * D:(h + 1) * D], r[:, h * D:(h + 1) * D],
                                AF.Identity, bias=nmrs[:, c:c + 1], scale=rstd[:, c:c + 1],
                            )
                            nc.tensor.transpose(tps[:, h * D:(h + 1) * D],
                                                nrm[:, h * D:(h + 1) * D], idn[:])
                        # one strided copy moves all 3 transposed heads out
                        dst = dstT[:, :, tt * 128:(tt + 1) * 128]
                        src = tps[:, 0:NH * D].rearrange("p (h t) -> p h t", h=NH)
                        nc.vector.tensor_copy(dst, src)

            # ---------------- stage 3+4: attention + output projection --------
            with (
                tc.tile_pool(name="att", bufs=3) as apool,
                tc.tile_pool(name="acc", bufs=2) as accpool,
                tc.tile_pool(name="ybuf", bufs=2) as ypool,
                tc.tile_pool(name="obuf", bufs=3) as opool,
            ):
                out_r = out.rearrange("(tt p) c -> p tt c", p=128)
                def emit_proj(qc, yTc):
                    # output projection for chunk qc's 4 token tiles
                    for j in range(QC // 128):
                        tt = qc * (QC // 128) + j
                        op0 = psA.tile([128, C // 2], F32, tag="ps")
                        op1 = psA.tile([128, C // 2], F32, tag="ps")
                        for h in range(NH):
                            lhs = yTc[:, h, j * 128:(j + 1) * 128]
                            nc.tensor.matmul(op0[:], lhs, wp_sb[:, h, 0:C // 2],
                                             start=(h == 0), stop=(h == NH - 1))
                            nc.tensor.matmul(op1[:], lhs, wp_sb[:, h, C // 2:C],
                                             start=(h == 0), stop=(h == NH - 1))
                        ot = opool.tile([128, C], F32, tag="ot")
                        nc.scalar.copy(ot[:, 0:C // 2], op0[:])
                        nc.scalar.copy(ot[:, C // 2:C], op1[:])
                        nc.sync.dma_start(out_r[:, tt, :], ot[:])

                pending = None
                for qc in range(NQC):
                    Q0 = qc * QC
                    n_st = (Q0 + QC) // 128
                    yTc = ypool.tile([128, NH, QC], F32R, tag="yT")  # [d, h, q]
                    for h in range(NH):
                        yps = psY.tile([128, QC], F32, tag="yps")
                        dps = psT.tile([128, QC], F32, tag="tp")
                        for st in range(n_st):
                            sps = psA.tile([128, QC], F32, tag="ps")
                            nc.tensor.matmul(
                                sps[:],
                                KT[:, h, st * 128:(st + 1) * 128],
                                QT[:, h, Q0:Q0 + QC],
                                start=True, stop=True,
                            )
                            et = apool.tile([128, QC], F32R, tag="et")
                            nc.scalar.activation(et[:], sps[:], AF.Exp, scale=SCALE)
                            if st * 128 >= Q0:  # diagonal block: zero where s > q
                                k = st - n_st + 4
                                nc.gpsimd.affine_select(
                                    et[:], et[:],
                                    pattern=[[1, QC]],
                                    compare_op=ALU.is_ge,
                                    fill=0.0,
                                    base=-(128 * k),
                                    channel_multiplier=-1,
                                )
                            nc.tensor.matmul(
                                yps[:],
                                V[:, st, h * D:(h + 1) * D],
                                et[:],
                                start=(st == 0), stop=(st == n_st - 1),
                                skip_group_check=True,
                            )
                            # softmax denominator on PE: accumulate ones^T @ exp
                            nc.tensor.matmul(
                                dps[:1, :],
                                ones[:],
                                et[:],
                                start=(st == 0), stop=(st == n_st - 1),
                                skip_group_check=True,
                            )
                        rc1 = accpool.tile([128, QC], F32, tag="rc1")
                        nc.vector.reciprocal(rc1[:1, :], dps[:1, :])
                        rbc = accpool.tile([128, QC], F32, tag="rbc")
                        nc.gpsimd.partition_broadcast(rbc[:], rc1[:1, :])
                        nc.vector.tensor_mul(yTc[:, h, :], yps[:], rbc[:])
                        if h == 0 and pending is not None:
                            # previous chunk's projection lands here so its
                            # yTc-normalize latency hides under this chunk's
                            # independent attention matmuls
                            emit_proj(*pending)
                            pending = None

                    pending = (qc, yTc)
                emit_proj(*pending)

    nc.compile()
    return nc


def _get_nc():
    if "nc" not in _CACHE:
        _CACHE["nc"] = _build_nc()
    return _CACHE["nc"]


def _in_maps(x, cos, sin, wq, wk, wv, wproj):
    cos3 = np.ascontiguousarray(np.tile(np.asarray(cos, np.float32), (1, NH)))
    sin3 = np.ascontiguousarray(np.tile(np.asarray(sin, np.float32), (1, NH)))
    ident = np.eye(128, dtype=np.float32)
    maps = []
    for c in range(8):
        b = c // 2
        hs = (c % 2) * NH
        sl = slice(hs * D, (hs + NH) * D)
        maps.append({
            "xT": np.ascontiguousarray(np.asarray(x[b], np.float32).T),
            "wqT": np.ascontiguousarray(np.asarray(wq, np.float32)[sl].T),
            "wkT": np.ascontiguousarray(np.asarray(wk, np.float32)[sl].T),
            "wvT": np.ascontiguousarray(np.asarray(wv, np.float32)[sl].T),
            "wpT": np.ascontiguousarray(np.asarray(wproj, np.float32).T[sl]),
            "cos3": cos3,
            "sin3": sin3,
            "ident": ident,
            "ones_in": np.ones((128, 1), dtype=np.float32),
        })
    return maps


def kernel(x, cos, sin, wq, wk, wv, wproj, _trace=False):
    nc = _get_nc()
    maps = _in_maps(x, cos, sin, wq, wk, wv, wproj)
    res = run_bass_kernel_spmd(nc, maps, core_ids=list(range(8)), trace=_trace)
    parts = [r["out"] for r in res.results]
    outv = np.stack([parts[2 * b] + parts[2 * b + 1] for b in range(B)]).astype(np.float32)
    if _trace:
        _CACHE["last_results"] = res
    return outv
